# revision 15
# baseline (speedup 1.0000x reference)
"""Bidirectional Mamba block on 8 Trainium2 NeuronCores.

Sharding: tensor-parallel over d_inner (2048 -> 256 per core). Each core:
  - computes its x/z in_proj slice from the full hidden (replicated, bf16)
  - depthwise causal conv via ACT shifted-scaled copies + DVE adds
  - x-projection partials -> AllReduce -> full (dt, B, C) per core
  - selective scan per (direction, batch, d-tile, state-index) with the
    hardware tensor_tensor_scan instruction (all elementwise on DVE --
    GpSimd shares SBUF ports with DVE and only slows it down)
  - gating + out_proj partial -> ReduceScatter over L-stripes
Host gathers the 8 L-stripes and transposes to (B, L, D_MODEL).

Pipeline: per-batch emission order so in_proj/conv/x-proj/AllReduce of
batch b+1 and out_proj/ReduceScatter of batch b overlap the DVE-bound
scan phase. PSUM budget: yacc [128,L] f32 (4 banks, ti-major n-loop),
shared 512-col pool (3 banks), ps96 (1 bank).

softplus is unavailable in the ACT tables; we use the exact identities
  G = sigmoid(-(x+b)),  delta = softplus(x+b) = -ln G,
  dA_n = exp(A_n * delta) = exp(-A_n * ln G)
so only Sigmoid/Ln/Exp/Silu/Copy LUTs are needed.
"""

from contextlib import ExitStack

import numpy as np
import ml_dtypes

B, L, DM = 2, 2048, 1024
DI, N, DCONV, R = 2048, 16, 4, 64
NCORES = 8
DC = DI // NCORES          # 256 d_inner channels per core
TD = DC // 128             # 2 d-tiles of 128
TC = 512                   # t-chunk for matmuls
NTC = L // TC
BF = ml_dtypes.bfloat16

_CACHE = {}


def _rev(ap):
    """Reverse the (single) free dim of a 2D [partition, free] AP."""
    import concourse.bass as bass
    n = ap.ap[-1][1]
    assert ap.ap[-1][0] == 1
    return bass.AP(ap.tensor, ap.offset + (n - 1),
                   [list(d) for d in ap.ap[:-1]] + [[-1, n]])


def _bcast(ap, parts=128):
    """Partition-broadcast a DRAM row (1D AP of length F) to (parts, F)."""
    import concourse.bass as bass
    assert ap.ap[-1][0] == 1
    return bass.AP(ap.tensor, ap.offset, [[0, parts], [1, ap.ap[-1][1]]])


def _build():
    import concourse.tile as tile
    from concourse import bacc, mybir
    from concourse.masks import make_identity

    f32, bf16 = mybir.dt.float32, mybir.dt.bfloat16
    fp16 = mybir.dt.float16
    AF = mybir.ActivationFunctionType
    OP = mybir.AluOpType

    nc = bacc.Bacc("TRN2", target_bir_lowering=False, debug=False,
                   num_devices=NCORES)

    # ---------------- DRAM parameters (per-core shards, host-prepped)
    hT = nc.declare_dram_parameter("hT", [DM, B * L], bf16, isOutput=False)
    win = nc.declare_dram_parameter("win", [DM, 4 * 128], bf16, isOutput=False)
    wx = nc.declare_dram_parameter("wx", [2, DC, 96], bf16, isOutput=False)
    wdt = nc.declare_dram_parameter("wdt", [2, R, DC], bf16, isOutput=False)
    wout = nc.declare_dram_parameter("wout", [DC, DM], bf16, isOutput=False)
    wconv = nc.declare_dram_parameter("wconv", [2, DCONV, DC], f32, isOutput=False)
    cbias = nc.declare_dram_parameter("cbias", [2, DC], f32, isOutput=False)
    bdt = nc.declare_dram_parameter("bdt", [2, DC], f32, isOutput=False)  # = -b_dt
    Dp = nc.declare_dram_parameter("Dp", [2, DC], f32, isOutput=False)
    Amat = nc.declare_dram_parameter("Amat", [2, DC, N], f32, isOutput=False)  # = -A
    out_p = nc.declare_dram_parameter("out", [B, DM, L // NCORES], bf16,
                                      isOutput=True)

    # ---------------- internal DRAM
    xdp = nc.dram_tensor("xdp", [2, B, 96, L], bf16)
    xdr = nc.dram_tensor("xdr", [2, B, 96, L], bf16, addr_space="Shared")
    zdram = nc.dram_tensor("zdram", [B * TD, 128, L], bf16)
    po = nc.dram_tensor("po", [B, 4, NCORES, DM // 4, L // NCORES], bf16)
    rso = nc.dram_tensor("rso", [B, 4, DM // 4, L // NCORES], bf16)

    with tile.TileContext(nc) as tc, ExitStack() as es:
        ws = es.enter_context(tc.tile_pool(name="weights", bufs=1))
        wx_sb = ws.tile([128, 2 * TD, 96], bf16)
        nc.sync.dma_start(out=wx_sb[:], in_=wx[:].rearrange("d (t p) m -> p (d t) m", p=128))
        wdt_sb = ws.tile([R, 2, DC], bf16)
        nc.sync.dma_start(out=wdt_sb[:], in_=wdt[:].rearrange("d k m -> k d m"))
        wout_sb = ws.tile([128, TD, DM], bf16)
        nc.sync.dma_start(out=wout_sb[:], in_=wout[:].rearrange("(t p) m -> p t m", p=128))
        wconv_sb = ws.tile([128, 2, DCONV, TD], f32)
        nc.sync.dma_start(out=wconv_sb[:], in_=wconv[:].rearrange("d k (t p) -> p d k t", p=128))
        cbias_sb = ws.tile([128, 2, TD], f32)
        nc.sync.dma_start(out=cbias_sb[:], in_=cbias[:].rearrange("d (t p) -> p d t", p=128))
        bdt_sb = ws.tile([128, 2, TD], f32)
        nc.sync.dma_start(out=bdt_sb[:], in_=bdt[:].rearrange("d (t p) -> p d t", p=128))
        Dp_sb = ws.tile([128, 2, TD], f32)
        nc.sync.dma_start(out=Dp_sb[:], in_=Dp[:].rearrange("d (t p) -> p d t", p=128))
        A_sb = ws.tile([128, 2, TD, N], f32)
        nc.sync.dma_start(out=A_sb[:], in_=Amat[:].rearrange("d (t p) n -> p d t n", p=128))
        win_sb = ws.tile([128, 8, 512], bf16)
        nc.sync.dma_start(out=win_sb[:], in_=win[:].rearrange("(k p) m -> p k m", p=128))
        idn = ws.tile([128, 128], fp16, name="idn")
        make_identity(nc, idn[:])
        # diag(D) stationaries: fold u*D into the yacc PSUM accumulation
        dD = {}
        for d in range(2):
            for ti in range(TD):
                t = ws.tile([128, 128], bf16, name=f"dD{d}{ti}")
                make_identity(nc, t[:])
                nc.scalar.activation(out=t[:], in_=t[:], func=AF.Copy,
                                     scale=Dp_sb[:, d, ti:ti + 1])
                dD[(d, ti)] = t

        # ---------------- persistent pools (whole-kernel lifetime)
        # comb: out_proj input, per (b, ti)
        act = es.enter_context(tc.tile_pool(name="acts", bufs=1))
        comb_sb = [act.tile([128, L], bf16, name=f"comb{g}") for g in range(B * TD)]
        # u tiles per (d, ti), double-buffered over b
        pu = es.enter_context(tc.tile_pool(name="pu", bufs=2))
        # x tiles per ti
        px = es.enter_context(tc.tile_pool(name="px", bufs=1))
        # in_proj staging
        p01 = es.enter_context(tc.tile_pool(name="p01", bufs=1))
        pcv = es.enter_context(tc.tile_pool(name="pcv", bufs=2))  # acc + shift tiles
        # phase-3 pools
        p3 = es.enter_context(tc.tile_pool(name="p3", bufs=1))
        plg = es.enter_context(tc.tile_pool(name="plg", bufs=1))
        pbc = es.enter_context(tc.tile_pool(name="pbc", bufs=2))
        pda = es.enter_context(tc.tile_pool(name="pda", bufs=3))
        pbu = es.enter_context(tc.tile_pool(name="pbu", bufs=2))
        pht = es.enter_context(tc.tile_pool(name="pht", bufs=4))
        phc = es.enter_context(tc.tile_pool(name="phc", bufs=3))
        p4s = es.enter_context(tc.tile_pool(name="p4s", bufs=2))
        # PSUM pools: yacc 4 banks, shared 512-col pool 3 banks, ps96 1 bank
        psy = es.enter_context(tc.tile_pool(name="psy", bufs=1, space="PSUM"))
        psm = es.enter_context(tc.tile_pool(name="psm", bufs=3, space="PSUM"))
        ps9 = es.enter_context(tc.tile_pool(name="ps9", bufs=1, space="PSUM"))

        u_sb = {}   # (b, d, ti) -> tile (tags (d,ti), bufs=2 over b)
        u_x = {}    # b -> x tiles

        def in_proj(b):
            """x/z in_proj for batch b -> x tiles + zdram; then conv -> u.

            b==0: x-parts first (second hT pass for z), conv on DVE (idle at
            head). b==1: single hT pass, conv via ACT copies + DVE adds (DVE
            is scan-saturated there)."""
            x_sb = [px.tile([128, L], bf16, name=f"x{ti}", tag=f"x{ti}")
                    for ti in range(TD)]
            u_x[b] = x_sb
            parts = (0, 1) if b == 0 else (0, 1, 2, 3)
            for tcn in range(NTC):
                hTc = p01.tile([128, 8, TC], bf16, name="hTc", tag="hTc")
                nc.sync.dma_start(
                    out=hTc[:],
                    in_=hT[:, b * L + tcn * TC: b * L + (tcn + 1) * TC]
                    .rearrange("(k p) c -> p k c", p=128))
                for part in parts:   # x-dt0, x-dt1, z-dt0, z-dt1
                    pst = psm.tile([128, TC], f32, name="pst", tag="mm512")
                    for k in range(8):
                        nc.tensor.matmul(pst[:], win_sb[:, k, part * 128:(part + 1) * 128],
                                         hTc[:, k, :], start=(k == 0), stop=(k == 7))
                    if part < TD:
                        nc.scalar.copy(
                            out=x_sb[part][:, tcn * TC:(tcn + 1) * TC], in_=pst[:])
                    else:
                        zt = p01.tile([128, TC], bf16, name="zt", tag="zt")
                        nc.scalar.activation(out=zt[:], in_=pst[:], func=AF.Silu)
                        nc.sync.dma_start(
                            out=zdram[b * TD + part - TD, :,
                                      tcn * TC:(tcn + 1) * TC],
                            in_=zt[:])
            if b != 0:
                conv(b)

        def z_pass(b):
            """Deferred z in_proj parts (b==0 head: keeps xproj/AR early)."""
            for tcn in range(NTC):
                hTc = p01.tile([128, 8, TC], bf16, name="hTc", tag="hTc")
                nc.sync.dma_start(
                    out=hTc[:],
                    in_=hT[:, b * L + tcn * TC: b * L + (tcn + 1) * TC]
                    .rearrange("(k p) c -> p k c", p=128))
                for part in (2, 3):
                    pst = psm.tile([128, TC], f32, name="pst", tag="mm512")
                    for k in range(8):
                        nc.tensor.matmul(pst[:], win_sb[:, k, part * 128:(part + 1) * 128],
                                         hTc[:, k, :], start=(k == 0), stop=(k == 7))
                    zt = p01.tile([128, TC], bf16, name="zt", tag="zt")
                    nc.scalar.activation(out=zt[:], in_=pst[:], func=AF.Silu)
                    nc.sync.dma_start(
                        out=zdram[b * TD + part - TD, :,
                                  tcn * TC:(tcn + 1) * TC],
                        in_=zt[:])

        def conv(b, dirs=(0, 1)):
            """Depthwise causal conv + silu -> u tiles."""
            if b == 0:   # DVE chain, chunked per t-block, tcn-major across ti
                # so each xproj input chunk (both ti) lands as early as possible.
                for d in dirs:
                    accs = [pcv.tile([128, L], bf16, name=f"cacc{ti}", tag="cacc")
                            for ti in range(TD)]
                    uts = [pu.tile([128, L], bf16, name=f"u{d}{ti}", tag=f"u{d}{ti}")
                           for ti in range(TD)]
                    for tcn in range(NTC):
                        lo, hi = tcn * TC, (tcn + 1) * TC
                        for ti in range(TD):
                            xs, acc, ut = u_x[b][ti], accs[ti], uts[ti]
                            nc.vector.tensor_scalar(
                                out=acc[:, lo:hi], in0=xs[:, lo:hi],
                                scalar1=wconv_sb[:, d, 3, ti:ti + 1],
                                scalar2=cbias_sb[:, d, ti:ti + 1],
                                op0=OP.mult, op1=OP.add)
                            if d == 0:
                                for k in (2, 1, 0):
                                    s = 3 - k
                                    a = max(lo, s)
                                    nc.vector.scalar_tensor_tensor(
                                        out=acc[:, a:hi], in0=xs[:, a - s:hi - s],
                                        scalar=wconv_sb[:, 0, k, ti:ti + 1],
                                        in1=acc[:, a:hi],
                                        op0=OP.mult, op1=OP.add)
                            else:
                                for m in (1, 2, 3):
                                    e2 = min(hi, L - m)
                                    nc.vector.scalar_tensor_tensor(
                                        out=acc[:, lo:e2], in0=xs[:, lo + m:e2 + m],
                                        scalar=wconv_sb[:, 1, 3 - m, ti:ti + 1],
                                        in1=acc[:, lo:e2],
                                        op0=OP.mult, op1=OP.add)
                            nc.scalar.activation(out=ut[:, lo:hi], in_=acc[:, lo:hi],
                                                 func=AF.Silu)
                    for ti in range(TD):
                        u_sb[(b, d, ti)] = uts[ti]
                return
            for d in dirs:
                for ti in range(TD):
                    xs = u_x[b][ti]
                    acc = pcv.tile([128, L], bf16, name="cacc", tag="cacc")
                    t0 = pcv.tile([128, L], bf16, name="ct0", tag="ct0")
                    t1 = pcv.tile([128, L], bf16, name="ct1", tag="ct0")
                    if d == 0:  # causal: acc[t] = sum_k w[k]*x[t-3+k] + cb
                        nc.scalar.activation(
                            out=acc[:], in_=xs[:], func=AF.Copy,
                            scale=wconv_sb[:, 0, 3, ti:ti + 1])
                        nc.scalar.activation(
                            out=t0[:, 1:], in_=xs[:, :L - 1], func=AF.Copy,
                            scale=wconv_sb[:, 0, 2, ti:ti + 1])
                        nc.scalar.activation(
                            out=t1[:, 2:], in_=xs[:, :L - 2], func=AF.Copy,
                            scale=wconv_sb[:, 0, 1, ti:ti + 1])
                        nc.vector.tensor_add(out=acc[:, 1:], in0=acc[:, 1:],
                                             in1=t0[:, 1:])
                        nc.scalar.activation(
                            out=t0[:, 3:], in_=xs[:, :L - 3], func=AF.Copy,
                            scale=wconv_sb[:, 0, 0, ti:ti + 1])
                        nc.vector.tensor_add(out=acc[:, 2:], in0=acc[:, 2:],
                                             in1=t1[:, 2:])
                        nc.vector.tensor_add(out=acc[:, 3:], in0=acc[:, 3:],
                                             in1=t0[:, 3:])
                    else:  # reverse dir in s-space: acc[s] = sum_m wr[3-m]*x[s+m] + cbr
                        nc.scalar.activation(
                            out=acc[:], in_=xs[:], func=AF.Copy,
                            scale=wconv_sb[:, 1, 3, ti:ti + 1])
                        nc.scalar.activation(
                            out=t0[:, :L - 1], in_=xs[:, 1:], func=AF.Copy,
                            scale=wconv_sb[:, 1, 2, ti:ti + 1])
                        nc.scalar.activation(
                            out=t1[:, :L - 2], in_=xs[:, 2:], func=AF.Copy,
                            scale=wconv_sb[:, 1, 1, ti:ti + 1])
                        nc.vector.tensor_add(out=acc[:, :L - 1], in0=acc[:, :L - 1],
                                             in1=t0[:, :L - 1])
                        nc.scalar.activation(
                            out=t0[:, :L - 3], in_=xs[:, 3:], func=AF.Copy,
                            scale=wconv_sb[:, 1, 0, ti:ti + 1])
                        nc.vector.tensor_add(out=acc[:, :L - 2], in0=acc[:, :L - 2],
                                             in1=t1[:, :L - 2])
                        nc.vector.tensor_add(out=acc[:, :L - 3], in0=acc[:, :L - 3],
                                             in1=t0[:, :L - 3])
                    ut = pu.tile([128, L], bf16, name=f"u{d}{ti}", tag=f"u{d}{ti}")
                    nc.scalar.activation(out=ut[:], in_=acc[:], func=AF.Silu,
                                         bias=cbias_sb[:, d, ti:ti + 1])
                    u_sb[(b, d, ti)] = ut

        def xproj_ar(b, d):
            """x-proj partials for (b, d) -> xdp -> AllReduce -> xdr."""
            for tcn in range(NTC):
                ps96 = ps9.tile([96, TC], f32, name="ps96", tag="ps96")
                for kt in range(TD):
                    nc.tensor.matmul(ps96[:], wx_sb[:, d * TD + kt, :],
                                     u_sb[(b, d, kt)][:, tcn * TC:(tcn + 1) * TC],
                                     start=(kt == 0), stop=(kt == TD - 1))
                sb96 = p01.tile([96, TC], bf16, name="sb96", tag="sb96")
                nc.scalar.copy(out=sb96[:], in_=ps96[:])
                nc.sync.dma_start(
                    out=xdp[d, b, :, tcn * TC:(tcn + 1) * TC], in_=sb96[:])
            nc.gpsimd.collective_compute(
                "AllReduce", OP.add, replica_groups=[list(range(NCORES))],
                ins=[xdp[d, b, 0:R].opt()], outs=[xdr[d, b, 0:R].opt()])
            nc.gpsimd.collective_compute(
                "AllReduce", OP.add, replica_groups=[list(range(NCORES))],
                ins=[xdp[d, b, R:96].opt()], outs=[xdr[d, b, R:96].opt()])

        def phase3(b, d, fillers=()):
            """dt/delta, scan, gating for (b, d). All elementwise on DVE.

            fillers: callbacks emitted spread across ti=0's n-loop (used to
            interleave the previous batch's out_proj PE work)."""
            dtT = p3.tile([R, L], bf16, name="dtT", tag="dtT")
            nc.sync.dma_start(out=dtT[:], in_=xdr[d, b, 0:R, :])
            lgs, dus = [], []
            for ti in range(TD):
                # G = sigmoid(-(dtproj + b_dt)); delta = -ln G
                lg = plg.tile([128, L], f32, name=f"lg{ti}", tag=f"lg{ti}")
                for tcn in range(NTC):
                    psd = psm.tile([128, TC], f32, name="psd", tag="mm512")
                    nc.tensor.matmul(psd[:], wdt_sb[:, d, ti * 128:(ti + 1) * 128],
                                     dtT[:, tcn * TC:(tcn + 1) * TC],
                                     start=True, stop=True)
                    nc.scalar.copy(out=lg[:, tcn * TC:(tcn + 1) * TC], in_=psd[:])
                lgs.append(lg)
            for ti in range(TD):   # batch per ACT table set: sigmoid x2, ln x2
                nc.scalar.activation(out=lgs[ti][:], in_=lgs[ti][:], func=AF.Sigmoid,
                                     scale=-1.0, bias=bdt_sb[:, d, ti:ti + 1])
            for ti in range(TD):
                nc.scalar.activation(out=lgs[ti][:], in_=lgs[ti][:], func=AF.Ln)
            for ti in range(TD):
                lg16 = plg.tile([128, L], fp16, name="lg16", tag="lg16")
                nc.scalar.activation(out=lg16[:], in_=lgs[ti][:], func=AF.Copy,
                                     scale=-1.0)
                du = plg.tile([128, L], fp16, name=f"du{ti}", tag=f"du{ti}")
                nc.vector.tensor_mul(out=du[:], in0=lg16[:],
                                     in1=u_sb[(b, d, ti)][:])
                dus.append(du)
            fi = 0
            for ti in range(TD):
                u3 = u_sb[(b, d, ti)]
                lg, du = lgs[ti], dus[ti]
                yacc = psy.tile([128, L], f32, name="yacc", tag="yacc")
                for n in range(N):
                    Bbc = pbc.tile([128, L], bf16, name="Bbc", tag="Bbc")
                    nc.sync.dma_start(out=Bbc[:], in_=_bcast(xdr[d, b, 64 + n, :]))
                    Cbc = pbc.tile([128, L], bf16, name="Cbc", tag="Cbc")
                    nc.sync.dma_start(out=Cbc[:], in_=_bcast(xdr[d, b, 80 + n, :]))
                    dA = pda.tile([128, L], fp16, name="dA", tag="dA")
                    nc.scalar.activation(out=dA[:], in_=lg[:], func=AF.Exp,
                                         scale=A_sb[:, d, ti, n:n + 1])
                    dBu = pbu.tile([128, L], fp16, name="dBu", tag="dBu")
                    nc.vector.tensor_mul(out=dBu[:], in0=du[:], in1=Bbc[:])
                    Ht = pht.tile([128, L], fp16, name="Ht", tag="Ht")
                    if d == 0:
                        nc.vector.tensor_tensor_scan(
                            out=Ht[:], data0=dA[:], data1=dBu[:], initial=0.0,
                            op0=OP.mult, op1=OP.add)
                    else:
                        nc.vector.tensor_tensor_scan(
                            out=_rev(Ht[:]), data0=_rev(dA[:]), data1=_rev(dBu[:]),
                            initial=0.0, op0=OP.mult, op1=OP.add)
                    Hc = phc.tile([128, L], fp16, name="Hc", tag="Hc")
                    nc.vector.tensor_mul(out=Hc[:], in0=Ht[:], in1=Cbc[:])
                    for ch in range(NTC):
                        nc.tensor.matmul(
                            yacc[:, ch * TC:(ch + 1) * TC], idn[:],
                            Hc[:, ch * TC:(ch + 1) * TC],
                            start=(n == 0), stop=False)
                    if ti == 0 and n % 2 == 1 and fi < len(fillers):
                        fillers[fi]()
                        fi += 1
                # y += D*u via diag-D stationary (closes the PSUM group)
                for ch in range(NTC):
                    nc.tensor.matmul(
                        yacc[:, ch * TC:(ch + 1) * TC], dD[(d, ti)][:],
                        u3[:, ch * TC:(ch + 1) * TC],
                        start=False, stop=True)
                # gating: comb = (u*D + y) * silu(z)   (zdram holds silu(z))
                g = b * TD + ti
                zt3 = p3.tile([128, L], bf16, name="zt3", tag="zt3")
                nc.sync.dma_start(out=zt3[:], in_=zdram[g])
                ycp = p3.tile([128, L], bf16, name="ycp", tag="ycp")
                nc.scalar.copy(out=ycp[:], in_=yacc[:])
                if d == 0:
                    nc.vector.tensor_mul(out=comb_sb[g][:], in0=ycp[:], in1=zt3[:])
                else:
                    yg = p3.tile([128, L], bf16, name="yg", tag="yg")
                    nc.vector.tensor_mul(out=yg[:], in0=ycp[:], in1=zt3[:])
                    nc.vector.tensor_add(out=comb_sb[g][:], in0=comb_sb[g][:],
                                         in1=yg[:])
            while fi < len(fillers):
                fillers[fi]()
                fi += 1

        def phase4_fillers(b, tail=False):
            """out_proj for batch b as per-mt closures + per-half RS."""
            LS = L // NCORES  # 256
            def mk_chunk(mt):
                def emit():
                    h, mtr = mt // 2, mt % 2
                    for tp in (0, 2):   # tcn pairs; kt-outer reuses stationary
                        psos = [psm.tile([128, TC], f32, name="pso", tag="mm512")
                                for _ in range(2)]
                        for kt in range(TD):
                            for j in range(2):
                                tcn = tp + j
                                nc.tensor.matmul(
                                    psos[j][:], wout_sb[:, kt, mt * 128:(mt + 1) * 128],
                                    comb_sb[b * TD + kt][:, tcn * TC:(tcn + 1) * TC],
                                    start=(kt == 0), stop=(kt == TD - 1))
                        for j in range(2):
                            tcn = tp + j
                            sbo = p4s.tile([128, TC], bf16, name="sbo", tag="sbo")
                            nc.scalar.copy(out=sbo[:], in_=psos[j][:])
                            for half in range(TC // LS):
                                r = tcn * (TC // LS) + half
                                nc.sync.dma_start(
                                    out=po[b, h, r, mtr * 128:(mtr + 1) * 128, :],
                                    in_=sbo[:, half * LS:(half + 1) * LS])
                return emit
            def mk_rs(h):
                def emit():
                    nc.gpsimd.collective_compute(
                        "ReduceScatter", OP.add,
                        replica_groups=[list(range(NCORES))],
                        ins=[po[b, h].opt()], outs=[rso[b, h].opt()])
                    nc.sync.dma_start(
                        out=out_p[b, h * (DM // 4):(h + 1) * (DM // 4)],
                        in_=rso[b, h])
                return emit
            if tail:
                # b=1 tail: fewer, bigger RS ops (CC trigger latency dominates)
                return [mk_chunk(0), mk_chunk(1), mk_chunk(2), mk_chunk(3),
                        mk_rs(0), mk_rs(1), mk_chunk(4), mk_chunk(5),
                        mk_chunk(6), mk_chunk(7), mk_rs(2), mk_rs(3)]
            return [mk_chunk(0), mk_chunk(1), mk_rs(0), mk_chunk(2), mk_chunk(3),
                    mk_rs(1), mk_chunk(4), mk_chunk(5), mk_rs(2), mk_chunk(6),
                    mk_chunk(7), mk_rs(3)]

        # ---------------- emission: per-batch pipeline
        in_proj(0)
        conv(0, dirs=(0,))
        xproj_ar(0, 0)
        conv(0, dirs=(1,))
        xproj_ar(0, 1)
        z_pass(0)
        phase3(0, 0)
        in_proj(1)
        for d in range(2):
            xproj_ar(1, d)
        phase3(0, 1)
        phase3(1, 0, fillers=phase4_fillers(0))
        phase3(1, 1)
        for f in phase4_fillers(1, tail=True):
            f()

    nc.compile()
    return nc


def _prep_inputs(inputs):
    """Host-side shard prep: returns in_maps (one dict per core)."""
    h = np.asarray(inputs["hidden"], np.float32)
    W_in = np.asarray(inputs["W_in"], np.float32)
    W_out = np.asarray(inputs["W_out"], np.float32)
    hT = np.ascontiguousarray(h.reshape(B * L, DM).T).astype(BF)

    def f32a(k):
        return np.asarray(inputs[k], np.float32)

    in_maps = []
    for c in range(NCORES):
        sl = slice(c * DC, (c + 1) * DC)
        win = np.concatenate([W_in[sl].T, W_in[DI + c * DC: DI + (c + 1) * DC].T],
                             axis=1)  # (1024, 512): x | z
        m = {
            "hT": hT,
            "win": win.astype(BF),
            "wx": np.stack([f32a("W_x_f")[:, sl].T, f32a("W_x_r")[:, sl].T]).astype(BF),
            "wdt": np.stack([f32a("W_dt_f")[sl].T,
                             f32a("W_dt_r")[sl].T]).astype(BF),
            "wout": W_out[:, sl].T.astype(BF),
            "wconv": np.stack([f32a("conv_w_f")[sl].T, f32a("conv_w_r")[sl].T]),
            "cbias": np.stack([f32a("conv_b_f")[sl], f32a("conv_b_r")[sl]]),
            "bdt": np.stack([-f32a("b_dt_f")[sl], -f32a("b_dt_r")[sl]]),
            "Dp": np.stack([f32a("D_f")[sl], f32a("D_r")[sl]]),
            "Amat": np.stack([np.exp(f32a("A_log_f")[sl]),
                              np.exp(f32a("A_log_r")[sl])]),
        }
        m = {k: np.ascontiguousarray(v) for k, v in m.items()}
        in_maps.append(m)
    return in_maps


def kernel(**inputs) -> np.ndarray:
    import time
    from concourse.bass_utils import run_bass_kernel_spmd
    if "nc" not in _CACHE:
        _CACHE["nc"] = _build()
    nc = _CACHE["nc"]
    in_maps = _prep_inputs(inputs)
    res = None
    for attempt in range(3):
        try:
            res = run_bass_kernel_spmd(nc, in_maps, list(range(NCORES))).results
            break
        except Exception:
            if attempt == 2:
                raise
            time.sleep(5)
    # res[c]["out"]: (B, DM, 256) covering t in [256c, 256c+256)
    stripes = np.stack([np.asarray(res[c]["out"], np.float32)
                        for c in range(NCORES)], axis=0)  # (8, B, DM, 256)
    out = stripes.transpose(1, 0, 3, 2).reshape(B, L, DM)
    return np.ascontiguousarray(out)


# revision 17
# speedup vs baseline: 1.1783x; 1.1783x over previous
"""Bidirectional Mamba block on 8 Trainium2 NeuronCores.

Sharding: tensor-parallel over d_inner (2048 -> 256 per core). Each core:
  - computes its x/z in_proj slice from the full hidden (replicated, bf16)
  - depthwise causal conv via ACT shifted-scaled copies + DVE adds
  - x-projection partials -> AllReduce -> full (dt, B, C) per core
  - selective scan per (direction, batch, d-tile, state-index) with the
    hardware tensor_tensor_scan instruction (all elementwise on DVE --
    GpSimd shares SBUF ports with DVE and only slows it down)
  - gating + out_proj partial -> ReduceScatter over L-stripes
Host gathers the 8 L-stripes and transposes to (B, L, D_MODEL).

Pipeline: per-batch emission order so in_proj/conv/x-proj/AllReduce of
batch b+1 and out_proj/ReduceScatter of batch b overlap the DVE-bound
scan phase. PSUM budget: yacc [128,L] f32 (4 banks, ti-major n-loop),
shared 512-col pool (3 banks), ps96 (1 bank).

softplus is unavailable in the ACT tables; we use the exact identities
  G = sigmoid(-(x+b)),  delta = softplus(x+b) = -ln G,
  dA_n = exp(A_n * delta) = exp(-A_n * ln G)
so only Sigmoid/Ln/Exp/Silu/Copy LUTs are needed.
"""

from contextlib import ExitStack

import numpy as np
import ml_dtypes

B, L, DM = 2, 2048, 1024
DI, N, DCONV, R = 2048, 16, 4, 64
NCORES = 8
DC = DI // NCORES          # 256 d_inner channels per core
TD = DC // 128             # 2 d-tiles of 128
TC = 512                   # t-chunk for matmuls
NTC = L // TC
BF = ml_dtypes.bfloat16

_CACHE = {}


def _rev(ap):
    """Reverse the (single) free dim of a 2D [partition, free] AP."""
    import concourse.bass as bass
    n = ap.ap[-1][1]
    assert ap.ap[-1][0] == 1
    return bass.AP(ap.tensor, ap.offset + (n - 1),
                   [list(d) for d in ap.ap[:-1]] + [[-1, n]])


def _bcast(ap, parts=128):
    """Partition-broadcast a DRAM row (1D AP of length F) to (parts, F)."""
    import concourse.bass as bass
    assert ap.ap[-1][0] == 1
    return bass.AP(ap.tensor, ap.offset, [[0, parts], [1, ap.ap[-1][1]]])


def _build():
    import concourse.tile as tile
    from concourse import bacc, mybir
    from concourse.masks import make_identity

    f32, bf16 = mybir.dt.float32, mybir.dt.bfloat16
    fp16 = mybir.dt.float16
    AF = mybir.ActivationFunctionType
    OP = mybir.AluOpType

    nc = bacc.Bacc("TRN2", target_bir_lowering=False, debug=False,
                   num_devices=NCORES)

    # ---------------- DRAM parameters (per-core shards, host-prepped)
    hT = nc.declare_dram_parameter("hT", [DM, B * L], bf16, isOutput=False)
    win = nc.declare_dram_parameter("win", [DM, 4 * 128], bf16, isOutput=False)
    wx = nc.declare_dram_parameter("wx", [2, DC, 96], bf16, isOutput=False)
    wdt = nc.declare_dram_parameter("wdt", [2, R, DC], bf16, isOutput=False)
    wout = nc.declare_dram_parameter("wout", [DC, DM], bf16, isOutput=False)
    wconv = nc.declare_dram_parameter("wconv", [2, DCONV, DC], f32, isOutput=False)
    cbias = nc.declare_dram_parameter("cbias", [2, DC], f32, isOutput=False)
    bdt = nc.declare_dram_parameter("bdt", [2, DC], f32, isOutput=False)  # = -b_dt
    Dp = nc.declare_dram_parameter("Dp", [2, DC], f32, isOutput=False)
    Amat = nc.declare_dram_parameter("Amat", [2, DC, N], f32, isOutput=False)  # = -A
    out_p = nc.declare_dram_parameter("out", [B, DM, L // NCORES], bf16,
                                      isOutput=True)

    # ---------------- internal DRAM
    xdp = nc.dram_tensor("xdp", [2, B, 96, L], bf16)
    xdr = nc.dram_tensor("xdr", [2, B, 96, L], bf16, addr_space="Shared")
    zdram = nc.dram_tensor("zdram", [B * TD, 128, L], bf16)
    po = nc.dram_tensor("po", [B, 4, NCORES, DM // 4, L // NCORES], bf16)
    rso = nc.dram_tensor("rso", [B, 4, DM // 4, L // NCORES], bf16)

    with tile.TileContext(nc) as tc, ExitStack() as es:
        ws = es.enter_context(tc.tile_pool(name="weights", bufs=1))
        wx_sb = ws.tile([128, 2 * TD, 96], bf16)
        nc.sync.dma_start(out=wx_sb[:], in_=wx[:].rearrange("d (t p) m -> p (d t) m", p=128))
        wdt_sb = ws.tile([R, 2, DC], bf16)
        nc.sync.dma_start(out=wdt_sb[:], in_=wdt[:].rearrange("d k m -> k d m"))
        wout_sb = ws.tile([128, TD, DM], bf16)
        nc.sync.dma_start(out=wout_sb[:], in_=wout[:].rearrange("(t p) m -> p t m", p=128))
        wconv_sb = ws.tile([128, 2, DCONV, TD], f32)
        nc.sync.dma_start(out=wconv_sb[:], in_=wconv[:].rearrange("d k (t p) -> p d k t", p=128))
        cbias_sb = ws.tile([128, 2, TD], f32)
        nc.sync.dma_start(out=cbias_sb[:], in_=cbias[:].rearrange("d (t p) -> p d t", p=128))
        bdt_sb = ws.tile([128, 2, TD], f32)
        nc.sync.dma_start(out=bdt_sb[:], in_=bdt[:].rearrange("d (t p) -> p d t", p=128))
        Dp_sb = ws.tile([128, 2, TD], f32)
        nc.sync.dma_start(out=Dp_sb[:], in_=Dp[:].rearrange("d (t p) -> p d t", p=128))
        A_sb = ws.tile([128, 2, TD, N], f32)
        nc.sync.dma_start(out=A_sb[:], in_=Amat[:].rearrange("d (t p) n -> p d t n", p=128))
        win_sb = ws.tile([128, 8, 512], bf16)
        nc.sync.dma_start(out=win_sb[:], in_=win[:].rearrange("(k p) m -> p k m", p=128))
        idn = ws.tile([128, 128], fp16, name="idn")
        make_identity(nc, idn[:])
        # diag(D) stationaries: fold u*D into the yacc PSUM accumulation
        dD = {}
        for d in range(2):
            for ti in range(TD):
                t = ws.tile([128, 128], bf16, name=f"dD{d}{ti}")
                make_identity(nc, t[:])
                nc.scalar.activation(out=t[:], in_=t[:], func=AF.Copy,
                                     scale=Dp_sb[:, d, ti:ti + 1])
                dD[(d, ti)] = t

        # ---------------- persistent pools (whole-kernel lifetime)
        # comb: out_proj input, per (b, ti)
        act = es.enter_context(tc.tile_pool(name="acts", bufs=1))
        comb_sb = [act.tile([128, L], bf16, name=f"comb{g}") for g in range(B * TD)]
        # u tiles per (d, ti), double-buffered over b
        pu = es.enter_context(tc.tile_pool(name="pu", bufs=2))
        # x tiles per ti
        px = es.enter_context(tc.tile_pool(name="px", bufs=1))
        # in_proj staging
        p01 = es.enter_context(tc.tile_pool(name="p01", bufs=1))
        pcv = es.enter_context(tc.tile_pool(name="pcv", bufs=2))  # acc + shift tiles
        # phase-3 pools
        p3 = es.enter_context(tc.tile_pool(name="p3", bufs=1))
        plg = es.enter_context(tc.tile_pool(name="plg", bufs=1))
        pbc = es.enter_context(tc.tile_pool(name="pbc", bufs=2))
        pda = es.enter_context(tc.tile_pool(name="pda", bufs=3))
        pbu = es.enter_context(tc.tile_pool(name="pbu", bufs=2))
        pht = es.enter_context(tc.tile_pool(name="pht", bufs=4))
        phc = es.enter_context(tc.tile_pool(name="phc", bufs=3))
        p4s = es.enter_context(tc.tile_pool(name="p4s", bufs=2))
        # PSUM pools: yacc 4 banks, shared 512-col pool 3 banks, ps96 1 bank
        psy = es.enter_context(tc.tile_pool(name="psy", bufs=1, space="PSUM"))
        psm = es.enter_context(tc.tile_pool(name="psm", bufs=3, space="PSUM"))
        ps9 = es.enter_context(tc.tile_pool(name="ps9", bufs=1, space="PSUM"))

        u_sb = {}   # (b, d, ti) -> tile (tags (d,ti), bufs=2 over b)
        u_x = {}    # b -> x tiles

        def in_proj(b):
            """x/z in_proj for batch b -> x tiles + zdram; then conv -> u.

            b==0: x-parts first (second hT pass for z), conv on DVE (idle at
            head). b==1: single hT pass, conv via ACT copies + DVE adds (DVE
            is scan-saturated there)."""
            x_sb = [px.tile([128, L], bf16, name=f"x{ti}", tag=f"x{ti}")
                    for ti in range(TD)]
            u_x[b] = x_sb
            parts = (0, 1) if b == 0 else (0, 1, 2, 3)
            for tcn in range(NTC):
                hTc = p01.tile([128, 8, TC], bf16, name="hTc", tag="hTc")
                nc.sync.dma_start(
                    out=hTc[:],
                    in_=hT[:, b * L + tcn * TC: b * L + (tcn + 1) * TC]
                    .rearrange("(k p) c -> p k c", p=128))
                for part in parts:   # x-dt0, x-dt1, z-dt0, z-dt1
                    pst = psm.tile([128, TC], f32, name="pst", tag="mm512")
                    for k in range(8):
                        nc.tensor.matmul(pst[:], win_sb[:, k, part * 128:(part + 1) * 128],
                                         hTc[:, k, :], start=(k == 0), stop=(k == 7))
                    if part < TD:
                        nc.scalar.copy(
                            out=x_sb[part][:, tcn * TC:(tcn + 1) * TC], in_=pst[:])
                    else:
                        zt = p01.tile([128, TC], bf16, name="zt", tag="zt")
                        nc.scalar.activation(out=zt[:], in_=pst[:], func=AF.Silu)
                        nc.sync.dma_start(
                            out=zdram[b * TD + part - TD, :,
                                      tcn * TC:(tcn + 1) * TC],
                            in_=zt[:])
            if b != 0:
                conv(b)

        def z_pass(b):
            """Deferred z in_proj parts (b==0 head: keeps xproj/AR early)."""
            for tcn in range(NTC):
                hTc = p01.tile([128, 8, TC], bf16, name="hTc", tag="hTc")
                nc.sync.dma_start(
                    out=hTc[:],
                    in_=hT[:, b * L + tcn * TC: b * L + (tcn + 1) * TC]
                    .rearrange("(k p) c -> p k c", p=128))
                for part in (2, 3):
                    pst = psm.tile([128, TC], f32, name="pst", tag="mm512")
                    for k in range(8):
                        nc.tensor.matmul(pst[:], win_sb[:, k, part * 128:(part + 1) * 128],
                                         hTc[:, k, :], start=(k == 0), stop=(k == 7))
                    zt = p01.tile([128, TC], bf16, name="zt", tag="zt")
                    nc.scalar.activation(out=zt[:], in_=pst[:], func=AF.Silu)
                    nc.sync.dma_start(
                        out=zdram[b * TD + part - TD, :,
                                  tcn * TC:(tcn + 1) * TC],
                        in_=zt[:])

        def conv(b, dirs=(0, 1)):
            """Depthwise causal conv + silu -> u tiles."""
            for d in dirs:
                for ti in range(TD):
                    xs = u_x[b][ti]
                    acc = pcv.tile([128, L], bf16, name="cacc", tag="cacc")
                    if b == 0:   # DVE chain, chunked per t-block so each
                        # chunk starts as soon as its x columns land; the
                        # d=0 silu chunks unblock xproj/AllReduce early.
                        ut = pu.tile([128, L], bf16, name=f"u{d}{ti}", tag=f"u{d}{ti}")
                        for tcn in range(NTC):
                            lo, hi = tcn * TC, (tcn + 1) * TC
                            nc.vector.tensor_scalar(
                                out=acc[:, lo:hi], in0=xs[:, lo:hi],
                                scalar1=wconv_sb[:, d, 3, ti:ti + 1],
                                scalar2=cbias_sb[:, d, ti:ti + 1],
                                op0=OP.mult, op1=OP.add)
                            if d == 0:
                                for k in (2, 1, 0):
                                    s = 3 - k
                                    a = max(lo, s)
                                    nc.vector.scalar_tensor_tensor(
                                        out=acc[:, a:hi], in0=xs[:, a - s:hi - s],
                                        scalar=wconv_sb[:, 0, k, ti:ti + 1],
                                        in1=acc[:, a:hi],
                                        op0=OP.mult, op1=OP.add)
                            else:
                                for m in (1, 2, 3):
                                    e2 = min(hi, L - m)
                                    nc.vector.scalar_tensor_tensor(
                                        out=acc[:, lo:e2], in0=xs[:, lo + m:e2 + m],
                                        scalar=wconv_sb[:, 1, 3 - m, ti:ti + 1],
                                        in1=acc[:, lo:e2],
                                        op0=OP.mult, op1=OP.add)
                            nc.scalar.activation(out=ut[:, lo:hi], in_=acc[:, lo:hi],
                                                 func=AF.Silu)
                        u_sb[(b, d, ti)] = ut
                        continue
                    t0 = pcv.tile([128, L], bf16, name="ct0", tag="ct0")
                    t1 = pcv.tile([128, L], bf16, name="ct1", tag="ct0")
                    if d == 0:  # causal: acc[t] = sum_k w[k]*x[t-3+k] + cb
                        nc.scalar.activation(
                            out=acc[:], in_=xs[:], func=AF.Copy,
                            scale=wconv_sb[:, 0, 3, ti:ti + 1])
                        nc.scalar.activation(
                            out=t0[:, 1:], in_=xs[:, :L - 1], func=AF.Copy,
                            scale=wconv_sb[:, 0, 2, ti:ti + 1])
                        nc.scalar.activation(
                            out=t1[:, 2:], in_=xs[:, :L - 2], func=AF.Copy,
                            scale=wconv_sb[:, 0, 1, ti:ti + 1])
                        nc.vector.tensor_add(out=acc[:, 1:], in0=acc[:, 1:],
                                             in1=t0[:, 1:])
                        nc.scalar.activation(
                            out=t0[:, 3:], in_=xs[:, :L - 3], func=AF.Copy,
                            scale=wconv_sb[:, 0, 0, ti:ti + 1])
                        nc.vector.tensor_add(out=acc[:, 2:], in0=acc[:, 2:],
                                             in1=t1[:, 2:])
                        nc.vector.tensor_add(out=acc[:, 3:], in0=acc[:, 3:],
                                             in1=t0[:, 3:])
                    else:  # reverse dir in s-space: acc[s] = sum_m wr[3-m]*x[s+m] + cbr
                        nc.scalar.activation(
                            out=acc[:], in_=xs[:], func=AF.Copy,
                            scale=wconv_sb[:, 1, 3, ti:ti + 1])
                        nc.scalar.activation(
                            out=t0[:, :L - 1], in_=xs[:, 1:], func=AF.Copy,
                            scale=wconv_sb[:, 1, 2, ti:ti + 1])
                        nc.scalar.activation(
                            out=t1[:, :L - 2], in_=xs[:, 2:], func=AF.Copy,
                            scale=wconv_sb[:, 1, 1, ti:ti + 1])
                        nc.vector.tensor_add(out=acc[:, :L - 1], in0=acc[:, :L - 1],
                                             in1=t0[:, :L - 1])
                        nc.scalar.activation(
                            out=t0[:, :L - 3], in_=xs[:, 3:], func=AF.Copy,
                            scale=wconv_sb[:, 1, 0, ti:ti + 1])
                        nc.vector.tensor_add(out=acc[:, :L - 2], in0=acc[:, :L - 2],
                                             in1=t1[:, :L - 2])
                        nc.vector.tensor_add(out=acc[:, :L - 3], in0=acc[:, :L - 3],
                                             in1=t0[:, :L - 3])
                    ut = pu.tile([128, L], bf16, name=f"u{d}{ti}", tag=f"u{d}{ti}")
                    nc.scalar.activation(out=ut[:], in_=acc[:], func=AF.Silu,
                                         bias=cbias_sb[:, d, ti:ti + 1])
                    u_sb[(b, d, ti)] = ut

        def xproj_ar(b, d):
            """x-proj partials for (b, d) -> xdp -> AllReduce -> xdr."""
            for tcn in range(NTC):
                ps96 = ps9.tile([96, TC], f32, name="ps96", tag="ps96")
                for kt in range(TD):
                    nc.tensor.matmul(ps96[:], wx_sb[:, d * TD + kt, :],
                                     u_sb[(b, d, kt)][:, tcn * TC:(tcn + 1) * TC],
                                     start=(kt == 0), stop=(kt == TD - 1))
                sb96 = p01.tile([96, TC], bf16, name="sb96", tag="sb96")
                nc.scalar.copy(out=sb96[:], in_=ps96[:])
                nc.sync.dma_start(
                    out=xdp[d, b, :, tcn * TC:(tcn + 1) * TC], in_=sb96[:])
            nc.gpsimd.collective_compute(
                "AllReduce", OP.add, replica_groups=[list(range(NCORES))],
                ins=[xdp[d, b, 0:R].opt()], outs=[xdr[d, b, 0:R].opt()])
            nc.gpsimd.collective_compute(
                "AllReduce", OP.add, replica_groups=[list(range(NCORES))],
                ins=[xdp[d, b, R:96].opt()], outs=[xdr[d, b, R:96].opt()])

        def phase3(b, d, fillers=()):
            """dt/delta, scan, gating for (b, d). All elementwise on DVE.

            fillers: callbacks emitted spread across ti=0's n-loop (used to
            interleave the previous batch's out_proj PE work)."""
            dtT = p3.tile([R, L], bf16, name="dtT", tag="dtT")
            nc.sync.dma_start(out=dtT[:], in_=xdr[d, b, 0:R, :])
            lgs, dus = [], []
            for ti in range(TD):
                # G = sigmoid(-(dtproj + b_dt)); delta = -ln G
                lg = plg.tile([128, L], f32, name=f"lg{ti}", tag=f"lg{ti}")
                for tcn in range(NTC):
                    psd = psm.tile([128, TC], f32, name="psd", tag="mm512")
                    nc.tensor.matmul(psd[:], wdt_sb[:, d, ti * 128:(ti + 1) * 128],
                                     dtT[:, tcn * TC:(tcn + 1) * TC],
                                     start=True, stop=True)
                    nc.scalar.copy(out=lg[:, tcn * TC:(tcn + 1) * TC], in_=psd[:])
                lgs.append(lg)
            for ti in range(TD):   # batch per ACT table set: sigmoid x2, ln x2
                nc.scalar.activation(out=lgs[ti][:], in_=lgs[ti][:], func=AF.Sigmoid,
                                     scale=-1.0, bias=bdt_sb[:, d, ti:ti + 1])
            for ti in range(TD):
                nc.scalar.activation(out=lgs[ti][:], in_=lgs[ti][:], func=AF.Ln)
            for ti in range(TD):
                du = plg.tile([128, L], fp16, name=f"du{ti}", tag=f"du{ti}")
                nc.vector.scalar_tensor_tensor(
                    out=du[:], in0=lgs[ti][:], scalar=-1.0, in1=u_sb[(b, d, ti)][:],
                    op0=OP.mult, op1=OP.mult)
                dus.append(du)
            fi = 0
            for ti in range(TD):
                u3 = u_sb[(b, d, ti)]
                lg, du = lgs[ti], dus[ti]
                yacc = psy.tile([128, L], f32, name="yacc", tag="yacc")
                for n in range(N):
                    Bbc = pbc.tile([128, L], bf16, name="Bbc", tag="Bbc")
                    nc.sync.dma_start(out=Bbc[:], in_=_bcast(xdr[d, b, 64 + n, :]))
                    Cbc = pbc.tile([128, L], bf16, name="Cbc", tag="Cbc")
                    nc.sync.dma_start(out=Cbc[:], in_=_bcast(xdr[d, b, 80 + n, :]))
                    dA = pda.tile([128, L], fp16, name="dA", tag="dA")
                    nc.scalar.activation(out=dA[:], in_=lg[:], func=AF.Exp,
                                         scale=A_sb[:, d, ti, n:n + 1])
                    dBu = pbu.tile([128, L], fp16, name="dBu", tag="dBu")
                    nc.vector.tensor_mul(out=dBu[:], in0=du[:], in1=Bbc[:])
                    Ht = pht.tile([128, L], fp16, name="Ht", tag="Ht")
                    if d == 0:
                        nc.vector.tensor_tensor_scan(
                            out=Ht[:], data0=dA[:], data1=dBu[:], initial=0.0,
                            op0=OP.mult, op1=OP.add)
                    else:
                        nc.vector.tensor_tensor_scan(
                            out=_rev(Ht[:]), data0=_rev(dA[:]), data1=_rev(dBu[:]),
                            initial=0.0, op0=OP.mult, op1=OP.add)
                    Hc = phc.tile([128, L], fp16, name="Hc", tag="Hc")
                    nc.vector.tensor_mul(out=Hc[:], in0=Ht[:], in1=Cbc[:])
                    for ch in range(NTC):
                        nc.tensor.matmul(
                            yacc[:, ch * TC:(ch + 1) * TC], idn[:],
                            Hc[:, ch * TC:(ch + 1) * TC],
                            start=(n == 0), stop=False)
                    if ti == 0 and n % 2 == 1 and fi < len(fillers):
                        fillers[fi]()
                        fi += 1
                # y += D*u via diag-D stationary (closes the PSUM group)
                for ch in range(NTC):
                    nc.tensor.matmul(
                        yacc[:, ch * TC:(ch + 1) * TC], dD[(d, ti)][:],
                        u3[:, ch * TC:(ch + 1) * TC],
                        start=False, stop=True)
                # gating: comb = (u*D + y) * silu(z)   (zdram holds silu(z))
                g = b * TD + ti
                zt3 = p3.tile([128, L], bf16, name="zt3", tag="zt3")
                nc.sync.dma_start(out=zt3[:], in_=zdram[g])
                if d == 0:
                    nc.vector.tensor_mul(out=comb_sb[g][:], in0=yacc[:], in1=zt3[:])
                else:
                    yg = p3.tile([128, L], bf16, name="yg", tag="yg")
                    nc.vector.tensor_mul(out=yg[:], in0=yacc[:], in1=zt3[:])
                    nc.vector.tensor_add(out=comb_sb[g][:], in0=comb_sb[g][:],
                                         in1=yg[:])
            while fi < len(fillers):
                fillers[fi]()
                fi += 1

        def phase4_fillers(b, tail=False):
            """out_proj for batch b as per-mt closures + per-half RS."""
            LS = L // NCORES  # 256
            def mk_chunk(mt):
                def emit():
                    h, mtr = mt // 2, mt % 2
                    for tcn in range(NTC):
                        pso = psm.tile([128, TC], f32, name="pso", tag="mm512")
                        for kt in range(TD):
                            nc.tensor.matmul(
                                pso[:], wout_sb[:, kt, mt * 128:(mt + 1) * 128],
                                comb_sb[b * TD + kt][:, tcn * TC:(tcn + 1) * TC],
                                start=(kt == 0), stop=(kt == TD - 1))
                        sbo = p4s.tile([128, TC], bf16, name="sbo", tag="sbo")
                        nc.scalar.copy(out=sbo[:], in_=pso[:])
                        for half in range(TC // LS):
                            r = tcn * (TC // LS) + half
                            nc.sync.dma_start(
                                out=po[b, h, r, mtr * 128:(mtr + 1) * 128, :],
                                in_=sbo[:, half * LS:(half + 1) * LS])
                return emit
            def mk_rs(h):
                def emit():
                    nc.gpsimd.collective_compute(
                        "ReduceScatter", OP.add,
                        replica_groups=[list(range(NCORES))],
                        ins=[po[b, h].opt()], outs=[rso[b, h].opt()])
                    nc.sync.dma_start(
                        out=out_p[b, h * (DM // 4):(h + 1) * (DM // 4)],
                        in_=rso[b, h])
                return emit
            if tail:
                # b=1 tail: fewer, bigger RS ops (CC trigger latency dominates)
                return [mk_chunk(0), mk_chunk(1), mk_chunk(2), mk_chunk(3),
                        mk_rs(0), mk_rs(1), mk_chunk(4), mk_chunk(5),
                        mk_chunk(6), mk_chunk(7), mk_rs(2), mk_rs(3)]
            return [mk_chunk(0), mk_chunk(1), mk_rs(0), mk_chunk(2), mk_chunk(3),
                    mk_rs(1), mk_chunk(4), mk_chunk(5), mk_rs(2), mk_chunk(6),
                    mk_chunk(7), mk_rs(3)]

        # ---------------- emission: per-batch pipeline
        in_proj(0)
        conv(0, dirs=(0,))
        xproj_ar(0, 0)
        conv(0, dirs=(1,))
        xproj_ar(0, 1)
        z_pass(0)
        phase3(0, 0)
        in_proj(1)
        for d in range(2):
            xproj_ar(1, d)
        phase3(0, 1)
        phase3(1, 0, fillers=phase4_fillers(0))
        phase3(1, 1)
        for f in phase4_fillers(1, tail=True):
            f()

    nc.compile()
    return nc


def _prep_inputs(inputs):
    """Host-side shard prep: returns in_maps (one dict per core)."""
    h = np.asarray(inputs["hidden"], np.float32)
    W_in = np.asarray(inputs["W_in"], np.float32)
    W_out = np.asarray(inputs["W_out"], np.float32)
    hT = np.ascontiguousarray(h.reshape(B * L, DM).T).astype(BF)

    def f32a(k):
        return np.asarray(inputs[k], np.float32)

    in_maps = []
    for c in range(NCORES):
        sl = slice(c * DC, (c + 1) * DC)
        win = np.concatenate([W_in[sl].T, W_in[DI + c * DC: DI + (c + 1) * DC].T],
                             axis=1)  # (1024, 512): x | z
        m = {
            "hT": hT,
            "win": win.astype(BF),
            "wx": np.stack([f32a("W_x_f")[:, sl].T, f32a("W_x_r")[:, sl].T]).astype(BF),
            "wdt": np.stack([f32a("W_dt_f")[sl].T,
                             f32a("W_dt_r")[sl].T]).astype(BF),
            "wout": W_out[:, sl].T.astype(BF),
            "wconv": np.stack([f32a("conv_w_f")[sl].T, f32a("conv_w_r")[sl].T]),
            "cbias": np.stack([f32a("conv_b_f")[sl], f32a("conv_b_r")[sl]]),
            "bdt": np.stack([-f32a("b_dt_f")[sl], -f32a("b_dt_r")[sl]]),
            "Dp": np.stack([f32a("D_f")[sl], f32a("D_r")[sl]]),
            "Amat": np.stack([np.exp(f32a("A_log_f")[sl]),
                              np.exp(f32a("A_log_r")[sl])]),
        }
        m = {k: np.ascontiguousarray(v) for k, v in m.items()}
        in_maps.append(m)
    return in_maps


def kernel(**inputs) -> np.ndarray:
    import time
    from concourse.bass_utils import run_bass_kernel_spmd
    if "nc" not in _CACHE:
        _CACHE["nc"] = _build()
    nc = _CACHE["nc"]
    in_maps = _prep_inputs(inputs)
    res = None
    for attempt in range(3):
        try:
            res = run_bass_kernel_spmd(nc, in_maps, list(range(NCORES))).results
            break
        except Exception:
            if attempt == 2:
                raise
            time.sleep(5)
    # res[c]["out"]: (B, DM, 256) covering t in [256c, 256c+256)
    stripes = np.stack([np.asarray(res[c]["out"], np.float32)
                        for c in range(NCORES)], axis=0)  # (8, B, DM, 256)
    out = stripes.transpose(1, 0, 3, 2).reshape(B, L, DM)
    return np.ascontiguousarray(out)
